# revision 13
# baseline (speedup 1.0000x reference)
"""LSEP loss kernel for Trainium2, data-parallel over 8 NeuronCores.

loss_i = log(1 + (sum_{t=0} exp(x)) * (sum_{t=1} exp(-x)));  output = mean_i.

Per-core (512 rows): a = x - BIG*t, S_neg = sum exp(a),
S_pos = sum exp(-a - BIG), loss = ln(1 + S_neg*S_pos).
Raw-bass pipeline (standalone wait_ge instructions — Tile's embedded
multi-wait sync fields overflow this toolchain's walrus sync-wait slots):
SP issues DMAs, DVE does mask arithmetic, ACT does exp with accum_out.
Double-buffered over 8 chunks of [128, 4096].
"""

from contextlib import ExitStack

import numpy as np
import concourse.bass as bass
import concourse.mybir as mybir
from concourse.bass_utils import run_bass_kernel_spmd

B, C = 4096, 8192
N_CORES = 8
ROWS = B // N_CORES  # 512 rows per core
P = 128
NPT = ROWS // P      # 4 partition tiles
FD = 4096            # free-dim chunk
NCH = C // FD        # 2 chunks per row-tile
NCHUNKS = NPT * NCH  # 8 total chunks, col index = p*NCH + ch
SLOTS = 2
BIG = 1024.0

F32 = mybir.dt.float32
I32 = mybir.dt.int32
AF = mybir.ActivationFunctionType


def build_bass():
    nc = bass.Bass()
    x = nc.declare_dram_parameter("inputs", [ROWS, C], F32, isOutput=False)
    t = nc.declare_dram_parameter("targets", [ROWS, C], I32, isOutput=False)
    loss = nc.declare_dram_parameter("loss", [P, NPT], F32, isOutput=True)

    with ExitStack() as ctx:
        def sb(name, shape, dt):
            return ctx.enter_context(nc.sbuf_tensor(name, shape, dt))

        xt = [sb(f"xt{i}", [P, FD], F32) for i in range(SLOTS)]
        tt = [sb(f"tt{i}", [P, FD], I32) for i in range(SLOTS)]
        tm = [sb(f"tm{i}", [P, FD], F32) for i in range(SLOTS)]
        aa = [sb(f"aa{i}", [P, FD], F32) for i in range(SLOTS)]
        scr = [sb(f"scr{i}", [P, FD], F32) for i in range(SLOTS)]
        snegs = sb("snegs", [P, NCHUNKS], F32)
        sposs = sb("sposs", [P, NCHUNKS], F32)
        neg_big = sb("neg_big", [P, 1], F32)
        ssum = sb("ssum", [P, 2 * NPT], F32)
        prod = sb("prod", [P, NPT], F32)
        loss_t = sb("loss_t", [P, NPT], F32)
        dma_done = ctx.enter_context(nc.semaphore())
        dve_done = ctx.enter_context(nc.semaphore())
        act_done = ctx.enter_context(nc.semaphore())
        out_done = ctx.enter_context(nc.semaphore())
        block = ctx.enter_context(nc.Block())

        def chunk_slice(i):
            p, ch = divmod(i, NCH)
            return slice(p * P, (p + 1) * P), slice(ch * FD, (ch + 1) * FD)

        @block.sync
        def _(sync):
            for i in range(NCHUNKS):
                s = i % SLOTS
                if i >= 1:
                    # issuer must observe prior completions before further
                    # incs of dma_done (HW sem-update ordering rule)
                    sync.wait_ge(dma_done, 32 * i)
                if i >= SLOTS:
                    # xt/tt[s] free once chunk i-SLOTS's DVE TT consumed them;
                    # aa[s] free once its second exp ran.
                    sync.wait_ge(dve_done, 1 + 2 * (i - SLOTS) + 2)
                    sync.wait_ge(act_done, 2 * (i - SLOTS) + 2)
                rows, cols = chunk_slice(i)
                sync.dma_start(out=xt[s][:, :], in_=x[rows, cols]).then_inc(
                    dma_done, 16
                )
                sync.dma_start(out=tt[s][:, :], in_=t[rows, cols]).then_inc(
                    dma_done, 16
                )
            sync.wait_ge(act_done, 2 * NCHUNKS + 1)
            sync.dma_start(out=loss[:, :], in_=loss_t[:, :]).then_inc(out_done, 16)
            sync.wait_ge(out_done, 16)

        @block.vector
        def _(vector):
            nc.vector.memset(neg_big[:, :], -BIG).then_inc(dve_done, 1)
            for i in range(NCHUNKS):
                s = i % SLOTS
                vector.wait_ge(dma_done, 32 * i + 32)
                if i >= SLOTS:
                    # aa[s] is still being read by chunk i-SLOTS's exps
                    vector.wait_ge(act_done, 2 * (i - SLOTS) + 2)
                nc.vector.tensor_scalar_mul(tm[s][:, :], tt[s][:, :], -BIG)
                nc.vector.drain()
                nc.vector.tensor_add(aa[s][:, :], xt[s][:, :], tm[s][:, :]).then_inc(
                    dve_done, 2
                )
            vector.wait_ge(act_done, 2 * NCHUNKS)
            for p in range(NPT):
                nc.vector.reduce_sum(
                    ssum[:, p : p + 1],
                    snegs[:, p * NCH : (p + 1) * NCH],
                    axis=mybir.AxisListType.X,
                )
                nc.vector.reduce_sum(
                    ssum[:, NPT + p : NPT + p + 1],
                    sposs[:, p * NCH : (p + 1) * NCH],
                    axis=mybir.AxisListType.X,
                )
            nc.vector.drain()
            nc.vector.tensor_mul(
                prod[:, :], ssum[:, 0:NPT], ssum[:, NPT : 2 * NPT]
            ).then_inc(dve_done, 1)

        @block.scalar
        def _(scalar):
            for i in range(NCHUNKS):
                s = i % SLOTS
                scalar.wait_ge(dve_done, 1 + 2 * i + 2)
                nc.scalar.activation(
                    scr[s][:, :], aa[s][:, :], AF.Exp,
                    accum_out=snegs[:, i : i + 1],
                ).then_inc(act_done, 1)
                nc.scalar.drain()
                nc.scalar.activation(
                    scr[s][:, :], aa[s][:, :], AF.Exp,
                    scale=-1.0, bias=neg_big[:, 0:1],
                    accum_out=sposs[:, i : i + 1],
                ).then_inc(act_done, 1)
                nc.scalar.drain()
            scalar.wait_ge(dve_done, 1 + 2 * NCHUNKS + 1)
            nc.scalar.activation(
                loss_t[:, :], prod[:, :], AF.Ln, bias=1.0
            ).then_inc(act_done, 1)

    return nc


_NC_CACHE = []


def _get_nc():
    if not _NC_CACHE:
        _NC_CACHE.append(build_bass())
    return _NC_CACHE[0]


def _run(inputs, targets, trace=False, **kw):
    nc = _get_nc()
    in_maps = [
        {
            "inputs": np.ascontiguousarray(inputs[i * ROWS : (i + 1) * ROWS]),
            "targets": np.ascontiguousarray(targets[i * ROWS : (i + 1) * ROWS]),
        }
        for i in range(N_CORES)
    ]
    res = run_bass_kernel_spmd(nc, in_maps, list(range(N_CORES)), trace=trace, **kw)
    # loss tensor is [partition, p_tile]; row r of this core's shard = p_tile*128 + partition
    losses = np.concatenate(
        [res.results[i]["loss"].T.reshape(-1) for i in range(N_CORES)]
    )
    out = np.float32(np.mean(losses.astype(np.float64)))
    return out, res


def kernel(inputs: np.ndarray, targets: np.ndarray) -> np.ndarray:
    out, _ = _run(np.asarray(inputs), np.asarray(targets))
    return out
